# revision 1
# baseline (speedup 1.0000x reference)
"""Multi-head attention (B=2, L=S=2048, D=1024, H=16, E=64) on 8 TRN2 cores.

Sharding: tensor-parallel over heads. Core c owns heads 2c, 2c+1, i.e. the
128-wide slice [c*128:(c+1)*128] of the QKV projection outputs and the
matching row-slice of Wo. Each core reads the full (host-pre-transposed)
queries/keys/values, computes its two heads' attention, and writes a full
[1024, 4096] fp32 partial of the output projection; the host sums the 8
partials, transposes back and adds bo.

On-chip layout (per core):
  XT   = X^T            [1024 dmodel, 4096 tok]   bf16 (DMA'd per k-tile)
  QT/KT = (XW)^T        [128 e', 4096 tok]        fp32 in SBUF
  V'_h = [V_h | 1]      [4096 s, 65]              bf16 (PE-transposed VT)
  scores^T (per s-tile) [128 s, 2*512 (h,l)]      fp32 PSUM (fp32r matmuls)
  P^T = exp(s/8)        [128 s, 2*512]            bf16 SBUF (ScalarE)
  PV: V'_h.T @ P^T_h -> [65, 512] PSUM accumulated over 16 s-tiles;
      row 64 is the softmax denominator (ones column).
  out-proj: Wo_c.T @ OT [128 d, 512 tok] fp32r -> DMA straight to DRAM.
"""

import numpy as np
import ml_dtypes

import concourse.bass as bass
import concourse.bacc as bacc
import concourse.mybir as mybir
from concourse.tile import TileContext
from concourse.bass_utils import run_bass_kernel_spmd

BF16 = mybir.dt.bfloat16
F32 = mybir.dt.float32
F32R = mybir.dt.float32r

B, L, D = 2, 2048, 1024
TOK = B * L              # 4096
H, E = 16, 64
NCORES = 8
E2 = 128                 # projection output dims per core (2 heads)
NKT = D // 128           # 8 k-tiles of the contraction
LQ = 512                 # l-quarter: query-token tile inside attention
NLQ = L // LQ            # 4 per batch
NST = L // 128           # 16 s-tiles (key tokens) per batch
HEADS = 2                # heads per core

_CACHED_NC = None
_IDENT = np.eye(128, dtype=ml_dtypes.bfloat16)


def _warrange(w):
    # [D, E2] -> [128, NKT*E2]: row p holds [w[kt*128+p, :] for kt]
    return np.ascontiguousarray(
        w.reshape(NKT, 128, E2).transpose(1, 0, 2).reshape(128, NKT * E2)
    ).astype(ml_dtypes.bfloat16)


def _f32r(ap):
    return ap.bitcast(F32R)


def build_nc():
    nc = bacc.Bacc("TRN2", target_bir_lowering=False)

    xt = {n: nc.declare_dram_parameter(f"x{n}_t", [D, TOK], BF16, isOutput=False)
          for n in ("q", "k", "v")}
    w = {n: nc.declare_dram_parameter(f"w{n}", [128, NKT * E2], BF16,
                                      isOutput=False)
         for n in ("q", "k", "v")}
    bias = {n: nc.declare_dram_parameter(f"b{n}", [E2, 1], F32, isOutput=False)
            for n in ("q", "k", "v")}
    wo = nc.declare_dram_parameter("wo", [E2, D], F32, isOutput=False)
    ident_in = nc.declare_dram_parameter("ident_in", [128, 128], BF16, isOutput=False)
    out_t = nc.declare_dram_parameter("out_t", [D, TOK], BF16, isOutput=True)

    with TileContext(nc) as tc:
        with (
            tc.tile_pool(name="const", bufs=1) as const,
            tc.tile_pool(name="persist", bufs=1) as persist,
            tc.tile_pool(name="xt_pool", bufs=10) as xt_pool,
            tc.tile_pool(name="pt_pool", bufs=16) as pt_pool,
            tc.tile_pool(name="norm_pool", bufs=4) as norm_pool,
            tc.tile_pool(name="out_pool", bufs=4) as out_pool,
            tc.tile_pool(name="sc_ps", bufs=2, space="PSUM") as sc_ps,
            tc.tile_pool(name="misc_ps", bufs=2, space="PSUM") as misc_ps,
            tc.tile_pool(name="pv_ps", bufs=2, space="PSUM") as pv_ps,
        ):
            # ---- constants / persistent tensors (loads emitted JIT below) ----
            ident = const.tile([128, 128], BF16, tag="ident")
            w_sb = {n: const.tile([128, NKT * E2], BF16, tag=f"w_{n}",
                                  name=f"w_{n}") for n in ("q", "k", "v")}
            b_sb = {}
            for n in ("q", "k", "v"):
                b_dma = const.tile([E2, 1], F32, tag=f"bdma_{n}", name=f"bdma_{n}")
                nc.gpsimd.dma_start(out=b_dma[:], in_=bias[n].ap())
                b_sb[n] = const.tile([E2, 1], F32, tag=f"b_{n}", name=f"b_{n}")
                nc.vector.tensor_copy(b_sb[n][:], b_dma[:])

            ones_f = const.tile([1, 64], F32, tag="ones_f")
            nc.vector.memset(ones_f[:], 1.0)
            ones_r = const.tile([1, 64], F32R, tag="ones_r")
            nc.vector.tensor_copy(ones_r[:], ones_f[:])
            warm = const.tile([1, 2], F32, tag="warm")
            nc.vector.memset(warm[:], 0.0)
            nc.scalar.activation(warm[:], warm[:],
                                 mybir.ActivationFunctionType.Exp)
            wo_f32 = const.tile([E2, D], F32, tag="wo_f32")
            wo_sb = const.tile([E2, D], F32R, tag="wo")

            qt_sbs = [[persist.tile([E2, LQ], F32R, tag=f"qt{b}_{t}",
                                    name=f"qt{b}_{t}") for t in range(4)]
                      for b in range(B)]
            kt_sbs = [[persist.tile([E2, LQ], F32R, tag=f"kt{b}_{t}",
                                    name=f"kt{b}_{t}") for t in range(4)]
                      for b in range(B)]
            vt_sbs = [[persist.tile([E2, 512], BF16, tag=f"vt{b}_{g}",
                                     name=f"vt{b}_{g}") for g in range(4)]
                      for b in range(B)]
            # V' per head: [128 spart, (16 stile, 65)] with col 64 == 1.0
            vp_sbs = [[[persist.tile([128, 4 * 65], BF16, tag=f"vp{b}_{h}_{g}",
                                      name=f"vp{b}_{h}_{g}") for g in range(4)]
                       for h in range(HEADS)] for b in range(B)]
            ot_sbs = [persist.tile([E2, L], F32R, tag=f"ot{b}", name=f"ot{b}")
                      for b in range(B)]
            otu_sbs = [[persist.tile([65, L], F32, tag=f"otu{b}_{h}",
                                     name=f"otu{b}_{h}")
                        for h in range(HEADS)] for b in range(B)]

            for b in range(B):
                for h in range(HEADS):
                    for g in range(4):
                        nc.vector.memset(vp_sbs[b][h][g][:], 1.0)

            proj_out = {"q": qt_sbs, "k": kt_sbs, "v": None}

            w_loaded = set()

            def project_loads(n, b):
                t0 = b * L
                if n not in w_loaded:
                    w_loaded.add(n)
                    nc.sync.dma_start(out=w_sb[n][:], in_=w[n].ap())
                xts = []
                for kt in range(NKT):
                    xtile = xt_pool.tile([128, L], BF16, tag="xt")
                    nc.sync.dma_start(
                        out=xtile[:],
                        in_=xt[n].ap()[kt * 128:(kt + 1) * 128, t0:t0 + L],
                    )
                    xts.append(xtile)
                return xts

            def project_mms(n, b, xts, tts=None):
                for tt in (tts if tts is not None else range(L // 512)):
                    acc = misc_ps.tile([128, 512], F32, tag="mps")
                    for kt in range(NKT):
                        nc.tensor.matmul(
                            acc[:],
                            lhsT=w_sb[n][:, kt * E2:(kt + 1) * E2],
                            rhs=xts[kt][:, tt * 512:(tt + 1) * 512],
                            start=(kt == 0),
                            stop=(kt == NKT - 1),
                        )
                    if n == "v":
                        dst = vt_sbs[b][tt][:]
                    else:
                        dst = proj_out[n][b][tt][:]
                    nc.vector.tensor_scalar_add(dst, acc[:], b_sb[n][:])

            def project(n, b):
                project_mms(n, b, project_loads(n, b))

            def transpose_v(b):
                """VT [128 e', s] -> V'_h [128 s, (stile, 65)] for batch b."""
                for st in range(NST):
                    g, r = st // 4, st % 4
                    tp = misc_ps.tile([128, 128], BF16, tag="mps")
                    nc.tensor.transpose(
                        tp[:], vt_sbs[b][g][:, r * 128:(r + 1) * 128], ident[:]
                    )
                    for h in range(HEADS):
                        nc.vector.tensor_copy(
                            vp_sbs[b][h][g][:, r * 65: r * 65 + 64],
                            tp[:, h * 64:(h + 1) * 64],
                        )

            def attention_core(b, lq):
                """scores -> exp -> PV for one (batch, l-quarter)."""
                pv = [pv_ps.tile([65, LQ], F32, tag="pv",
                                 name=f"pv_{b}_{lq}_{h}") for h in range(HEADS)]
                qt = qt_sbs[b][lq]
                for st in range(NST):
                    r0 = (st % 4) * 128
                    kt = kt_sbs[b][st // 4]
                    sc = sc_ps.tile([128, 2 * LQ], F32, tag="sc")
                    for h in range(HEADS):
                        nc.tensor.matmul(
                            sc[:, h * LQ:(h + 1) * LQ],
                            lhsT=kt[h * 64:(h + 1) * 64, r0:r0 + 128],
                            rhs=qt[h * 64:(h + 1) * 64, :],
                            start=True, stop=True,
                        )
                    pt = pt_pool.tile([128, 2 * LQ], BF16, tag="pt")
                    nc.scalar.activation(
                        pt[:], sc[:], mybir.ActivationFunctionType.Exp,
                        scale=0.125,
                    )
                    for h in range(HEADS):
                        nc.tensor.matmul(
                            pv[h][:],
                            lhsT=vp_sbs[b][h][st // 4][:, (st % 4) * 65:
                                                       (st % 4) * 65 + 65],
                            rhs=pt[:, h * LQ:(h + 1) * LQ],
                            start=(st == 0), stop=(st == NST - 1),
                        )
                return pv

            def attention_tail(b, lq, pv):
                """pv drain, normalize, out-projection for one unit."""
                l0 = lq * LQ
                for h in range(HEADS):
                    nc.vector.tensor_copy(
                        otu_sbs[b][h][:, l0:l0 + LQ], pv[h][:]
                    )
                for h in range(HEADS):
                    rec = norm_pool.tile([1, LQ], F32R, tag="rec")
                    with nc.allow_low_precision(reason="f32r softmax denom"):
                        nc.vector.reciprocal(
                            rec[:], otu_sbs[b][h][64:65, l0:l0 + LQ]
                        )
                    bcp = misc_ps.tile([64, LQ], F32, tag="mps")
                    nc.tensor.matmul(bcp[:], lhsT=ones_r[:], rhs=rec[:],
                                     start=True, stop=True)
                    nc.vector.tensor_tensor(
                        out=ot_sbs[b][h * 64:(h + 1) * 64, l0:l0 + LQ],
                        in0=otu_sbs[b][h][0:64, l0:l0 + LQ],
                        in1=bcp[:],
                        op=mybir.AluOpType.mult,
                    )
                t0 = b * L
                for dt in range(D // 128):
                    op = misc_ps.tile([128, 512], F32, tag="mps")
                    nc.tensor.matmul(
                        op[:],
                        lhsT=wo_sb[:, dt * 128:(dt + 1) * 128],
                        rhs=ot_sbs[b][:, l0:l0 + LQ],
                        start=True, stop=True,
                    )
                    ob = out_pool.tile([128, 512], BF16, tag="ob")
                    nc.vector.tensor_copy(ob[:], op[:])
                    nc.scalar.dma_start(
                        out=out_t.ap()[dt * 128:(dt + 1) * 128,
                                       t0 + l0: t0 + l0 + LQ],
                        in_=ob[:],
                    )

            def load_wo():
                nc.gpsimd.dma_start(out=wo_f32[:], in_=wo.ap())
                nc.vector.tensor_copy(wo_sb[:], wo_f32[:])
                nc.gpsimd.dma_start(out=ident[:], in_=ident_in.ap())

            # schedule: b0 projections; attention units with one-unit-lagged
            # tails; b1 projections interleaved into b0's attention windows.
            project("k", 0)
            project("q", 0)
            load_wo()
            project("v", 0)
            transpose_v(0)
            b1_xts = {n: project_loads(n, 1) for n in ("q", "k", "v")}
            units = [(0, lq) for lq in range(NLQ)] + [(1, lq) for lq in range(NLQ)]
            prev = None
            for i, (b, lq) in enumerate(units):
                pv = attention_core(b, lq)
                if prev is not None:
                    attention_tail(*prev)
                prev = (b, lq, pv)
                if i == 1:
                    project_mms("q", 1, b1_xts["q"], (0, 1))
                elif i == 2:
                    project_mms("q", 1, b1_xts["q"], (2, 3))
                    project_mms("k", 1, b1_xts["k"], (0, 1))
                elif i == 3:
                    project_mms("k", 1, b1_xts["k"], (2, 3))
                    project_mms("v", 1, b1_xts["v"])
                    transpose_v(1)
            attention_tail(*prev)

    nc.compile()
    return nc


def _get_nc():
    global _CACHED_NC
    if _CACHED_NC is None:
        _CACHED_NC = build_nc()
    return _CACHED_NC


def _prep_inputs(queries, keys, values, Wq, bq, Wk, bk, Wv, bv, Wo, bo):
    bf16 = ml_dtypes.bfloat16
    x_t = {}
    for n, arr in (("q", queries), ("k", keys), ("v", values)):
        x_t[n] = np.ascontiguousarray(
            np.asarray(arr, np.float32).reshape(TOK, D).T
        ).astype(bf16)
    in_maps = []
    for c in range(NCORES):
        sl = slice(c * E2, (c + 1) * E2)
        m = {
            "xq_t": x_t["q"], "xk_t": x_t["k"], "xv_t": x_t["v"],
            "wq": _warrange(np.asarray(Wq, np.float32)[:, sl]),
            "wk": _warrange(np.asarray(Wk, np.float32)[:, sl]),
            "wv": _warrange(np.asarray(Wv, np.float32)[:, sl]),
            "bq": np.ascontiguousarray(np.asarray(bq, np.float32)[sl].reshape(E2, 1)),
            "bk": np.ascontiguousarray(np.asarray(bk, np.float32)[sl].reshape(E2, 1)),
            "bv": np.ascontiguousarray(np.asarray(bv, np.float32)[sl].reshape(E2, 1)),
            "wo": np.ascontiguousarray(np.asarray(Wo, np.float32)[sl, :]),
            "ident_in": _IDENT,
        }
        in_maps.append(m)
    return in_maps


def _postprocess(results, bo):
    acc = np.zeros((D, TOK), np.float64)
    for r in results:
        acc += r["out_t"].astype(np.float64)  # bf16 partials, summed in fp64
    out = acc.T.astype(np.float32) + np.asarray(bo, np.float32)[None, :]
    return out.reshape(B, L, D)


def run(trace=False, **inputs):
    nc = _get_nc()
    in_maps = _prep_inputs(**inputs)
    res = run_bass_kernel_spmd(nc, in_maps, core_ids=list(range(NCORES)),
                               trace=trace)
    out = _postprocess(res.results, inputs["bo"])
    return out, res


def kernel(**inputs):
    out, _ = run(trace=False, **inputs)
    return out



# revision 8
# speedup vs baseline: 1.1271x; 1.1271x over previous
"""Multi-head attention (B=2, L=S=2048, D=1024, H=16, E=64) on 8 TRN2 cores.

Sharding: tensor-parallel over heads. Core c owns heads 2c, 2c+1, i.e. the
128-wide slice [c*128:(c+1)*128] of the QKV projection outputs and the
matching row-slice of Wo. Each core reads the full (host-pre-transposed)
queries/keys/values, computes its two heads' attention, and writes a full
[1024, 4096] bf16 partial of the output projection; the host sums the 8
partials, transposes back and adds bo.

v2: software-pipelined emission built around the scalar-engine exp being
the pacing engine (~133us of exp vs ~140us of matmul):
  X^T  host-supplied as [128, 8 kt, 4096 tok] bf16; DMA'd per 512-tok chunk.
  QT/KT = (XW)^T        [128 e', 512]  f32r SBUF (4 tiles per batch each)
  V'_h = [V_h | 1]      [128 s, 4*65]  bf16 (PE-transposed VT)
  scores^T (per s-tile) [128 s, 2*512 (h,l)] f32 PSUM
  P^T = exp(s/8)        [128 s, 2*512] bf16 SBUF (ScalarE)
  PV flipped: lhsT=P^T chunk [128 s, 128 l], rhs=V'_h [128 s, 65]
      -> pv[h] [128 l, 4x(65 pad to 128)] PSUM accumulated over 16 s-tiles;
      col 64 of each chunk is the softmax denominator (ones column of V').
      Free-dim is 65 instead of 512, halving PE time for PV.
  normalize: per-partition reciprocal (DVE) + tensor_scalar mult (Pool)
      -> O [128 l, 128 e'] bf16; PE-transpose -> OT [128 e', 512] bf16.
  out-proj: Wo_c.T @ OT -> [128 d, 512] PSUM -> Pool copy into ob
      [128, 8, 512] bf16 -> one batched 3D DMA per unit (DVE queue).
DMA issue is kept off the scalar queue entirely so exp dispatch never
stalls behind descriptor generation. Units are emitted as 4 quads of
(8 score matmuls + 4 exps) each, with fill work (PV waves, projections,
V-transposes, previous unit's tail) interleaved between quads in an order
matched to DMA arrival times.
"""

import numpy as np
import ml_dtypes

import concourse.bass as bass
import concourse.bacc as bacc
import concourse.mybir as mybir
from concourse.tile import TileContext
from concourse.bass_utils import run_bass_kernel_spmd

BF16 = mybir.dt.bfloat16
F32 = mybir.dt.float32
F32R = mybir.dt.float32r

B, L, D = 2, 2048, 1024
TOK = B * L              # 4096
H, E = 16, 64
NCORES = 8
E2 = 128                 # projection output dims per core (2 heads)
NKT = D // 128           # 8 k-tiles of the contraction
LQ = 512                 # l-quarter: query-token tile inside attention
NLQ = L // LQ            # 4 per batch
NST = L // 128           # 16 s-tiles (key tokens) per batch
HEADS = 2                # heads per core
NDT = D // 128           # 8 output-row tiles

_CACHED_NC = None
_IDENT = np.eye(128, dtype=ml_dtypes.bfloat16)


def _warrange(w):
    # [D, E2] -> [128, NKT*E2]: row p holds [w[kt*128+p, :] for kt]
    return np.ascontiguousarray(
        w.reshape(NKT, 128, E2).transpose(1, 0, 2).reshape(128, NKT * E2)
    ).astype(ml_dtypes.bfloat16)


def build_nc():
    nc = bacc.Bacc("TRN2", target_bir_lowering=False)

    xt = {n: nc.declare_dram_parameter(f"x{n}_t", [128, NKT, TOK], BF16,
                                       isOutput=False)
          for n in ("q", "k", "v")}
    w = {n: nc.declare_dram_parameter(f"w{n}", [128, NKT * E2], BF16,
                                      isOutput=False)
         for n in ("q", "k", "v")}
    bias = {n: nc.declare_dram_parameter(f"b{n}", [E2, 1], F32, isOutput=False)
            for n in ("q", "k", "v")}
    wo = nc.declare_dram_parameter("wo", [E2, D], BF16, isOutput=False)
    ident_in = nc.declare_dram_parameter("ident_in", [128, 128], BF16,
                                         isOutput=False)
    out_t = nc.declare_dram_parameter("out_t", [128, NDT, TOK], BF16,
                                      isOutput=True)

    units = [(0, lq) for lq in range(NLQ)] + [(1, lq) for lq in range(NLQ)]

    with TileContext(nc) as tc:
        with (
            tc.tile_pool(name="const", bufs=1) as const,
            tc.tile_pool(name="persist", bufs=1) as persist,
            tc.tile_pool(name="xt_pool", bufs=10) as xt_pool,
            tc.tile_pool(name="pt_pool", bufs=12) as pt_pool,
            tc.tile_pool(name="o_pool", bufs=8) as o_pool,
            tc.tile_pool(name="rec_pool", bufs=16) as rec_pool,
            tc.tile_pool(name="ot_pool", bufs=2) as ot_pool,
            tc.tile_pool(name="ob_pool", bufs=2) as ob_pool,
            tc.tile_pool(name="sc_ps", bufs=2, space="PSUM") as sc_ps,
            tc.tile_pool(name="pv_ps", bufs=2, space="PSUM") as pv_ps,
            tc.tile_pool(name="misc_ps", bufs=1, space="PSUM") as misc_ps,
        ):
            # ---- constants ----
            ident = const.tile([128, 128], BF16, tag="ident")
            w_sb = {n: const.tile([128, NKT * E2], BF16, tag=f"w_{n}",
                                  name=f"w_{n}") for n in ("q", "k", "v")}
            b_sb = {}
            for n in ("q", "k", "v"):
                b_dma = const.tile([E2, 1], F32, tag=f"bdma_{n}",
                                   name=f"bdma_{n}")
                nc.gpsimd.dma_start(out=b_dma[:], in_=bias[n].ap())
                b_sb[n] = const.tile([E2, 1], F32, tag=f"b_{n}", name=f"b_{n}")
                nc.vector.tensor_copy(b_sb[n][:], b_dma[:])

            warm = const.tile([1, 2], F32, tag="warm")
            nc.vector.memset(warm[:], 0.0)
            nc.scalar.activation(warm[:], warm[:],
                                 mybir.ActivationFunctionType.Exp)
            wo_sb = const.tile([E2, D], BF16, tag="wo")

            qt_sbs = [[persist.tile([E2, LQ], F32R, tag=f"qt{b}_{t}",
                                    name=f"qt{b}_{t}") for t in range(4)]
                      for b in range(B)]
            kt_sbs = [[persist.tile([E2, LQ], F32R, tag=f"kt{b}_{t}",
                                    name=f"kt{b}_{t}") for t in range(4)]
                      for b in range(B)]
            vt_sbs = [[persist.tile([E2, 512], BF16, tag=f"vt{b}_{g}",
                                    name=f"vt{b}_{g}") for g in range(4)]
                      for b in range(B)]
            # V' per head: [128 spart, (4 stile, 65)] with col 64 == 1.0
            vp_sbs = [[[persist.tile([128, 4 * 65], BF16, tag=f"vp{b}_{h}_{g}",
                                     name=f"vp{b}_{h}_{g}") for g in range(4)]
                       for h in range(HEADS)] for b in range(B)]

            for b in range(B):
                for h in range(HEADS):
                    for g in range(4):
                        nc.vector.memset(vp_sbs[b][h][g][:], 1.0)

            nc.gpsimd.dma_start(out=wo_sb[:], in_=wo.ap())
            nc.gpsimd.dma_start(out=ident[:], in_=ident_in.ap())

            proj_out = {"q": qt_sbs, "k": kt_sbs}

            # ---- DMA issue (SP queue), in arrival-need order ----
            x_sb = {}

            def dma_w(n):
                nc.sync.dma_start(out=w_sb[n][:], in_=w[n].ap())

            def dma_x(n, b, tt):
                t0 = b * L + tt * 512
                xtile = xt_pool.tile([128, NKT, 512], BF16, tag="xt",
                                     name=f"x_{n}{b}_{tt}")
                nc.sync.dma_start(out=xtile[:],
                                  in_=xt[n].ap()[:, :, t0:t0 + 512])
                x_sb[(n, b, tt)] = xtile

            dma_w("k")
            dma_w("q")
            dma_x("k", 0, 0)
            dma_x("q", 0, 0)
            dma_w("v")
            dma_x("k", 0, 1)
            dma_x("v", 0, 0)
            dma_x("k", 0, 2)
            dma_x("v", 0, 1)
            dma_x("k", 0, 3)
            dma_x("q", 0, 1)
            dma_x("v", 0, 2)
            dma_x("v", 0, 3)
            dma_x("q", 0, 2)
            dma_x("q", 0, 3)
            for key in (("k", 1, 0), ("k", 1, 1), ("k", 1, 2), ("k", 1, 3),
                        ("q", 1, 0), ("v", 1, 0), ("v", 1, 1), ("q", 1, 1),
                        ("v", 1, 2), ("v", 1, 3), ("q", 1, 2), ("q", 1, 3)):
                dma_x(*key)

            # ---- emission helpers ----
            def proj_tt(n, b, tt):
                """Project one 512-token chunk: 8 accumulating matmuls."""
                acc = misc_ps.tile([128, 512], F32, tag="op",
                                   name=f"proj_{n}{b}_{tt}")
                xtile = x_sb[(n, b, tt)]
                for kt in range(NKT):
                    nc.tensor.matmul(
                        acc[:],
                        lhsT=w_sb[n][:, kt * E2:(kt + 1) * E2],
                        rhs=xtile[:, kt, :],
                        start=(kt == 0),
                        stop=(kt == NKT - 1),
                    )
                if n == "v":
                    dst = vt_sbs[b][tt][:]
                else:
                    dst = proj_out[n][b][tt][:]
                nc.vector.tensor_scalar_add(dst, acc[:], b_sb[n][:])

            def vtr(b, g):
                """VT [128 e', s] -> V'_h [128 s, (stile, 65)] for group g."""
                for r in range(4):
                    tp = misc_ps.tile([128, 128], BF16, tag="tr",
                                      name=f"vtr_{b}_{g}_{r}")
                    nc.tensor.transpose(
                        tp[:], vt_sbs[b][g][:, r * 128:(r + 1) * 128],
                        ident[:],
                    )
                    for h in range(HEADS):
                        nc.vector.tensor_copy(
                            vp_sbs[b][h][g][:, r * 65:r * 65 + 64],
                            tp[:, h * 64:(h + 1) * 64],
                        )

            pv_tiles = {}   # u -> [h tiles]
            pt_tiles = {}   # (u, st) -> tile

            def sc_quad(u, g):
                """Scores + exp for s-tiles 4g..4g+3 of unit u."""
                b, lq = units[u]
                if g == 0:
                    pv_tiles[u] = [
                        pv_ps.tile([128, 512], F32, tag="pv",
                                   name=f"pv_{u}_{h}") for h in range(HEADS)]
                qt = qt_sbs[b][lq]
                kt = kt_sbs[b][g]
                for r in range(4):
                    st = g * 4 + r
                    sc = sc_ps.tile([128, 2 * LQ], F32, tag="sc",
                                    name=f"sc_{u}_{st}")
                    for h in range(HEADS):
                        nc.tensor.matmul(
                            sc[:, h * LQ:(h + 1) * LQ],
                            lhsT=kt[h * 64:(h + 1) * 64,
                                    r * 128:(r + 1) * 128],
                            rhs=qt[h * 64:(h + 1) * 64, :],
                            start=True, stop=True,
                        )
                    pt = pt_pool.tile([128, 2 * LQ], BF16, tag="pt",
                                      name=f"pt_{u}_{st}")
                    nc.scalar.activation(
                        pt[:], sc[:], mybir.ActivationFunctionType.Exp,
                        scale=0.125,
                    )
                    pt_tiles[(u, st)] = pt

            def pv_wave(u, g, half):
                """Flipped PV matmuls for s-tiles of quad g (half 0/1)."""
                b, lq = units[u]
                pv = pv_tiles[u]
                for r in (0, 1) if half == 0 else (2, 3):
                    st = g * 4 + r
                    pt = pt_tiles[(u, st)]
                    for h in range(HEADS):
                        for c in range(4):
                            # start resets the whole 2KB PSUM bank, so only
                            # the first write of each head-tile may set it.
                            nc.tensor.matmul(
                                pv[h][:, c * 128:c * 128 + 65],
                                lhsT=pt[:, h * LQ + c * 128:
                                        h * LQ + (c + 1) * 128],
                                rhs=vp_sbs[b][h][g][:, r * 65:(r + 1) * 65],
                                start=(st == 0 and c == 0),
                                stop=(st == NST - 1 and c == 3),
                                skip_group_check=True,
                            )

            ot_tiles = {}
            ob_tiles = {}

            def tail_a(u):
                """Reciprocals, normalize, O transpose for unit u."""
                pv = pv_tiles[u]
                rec_hc = [[rec_pool.tile([128, 1], F32, tag="rec",
                                         name=f"rec_{u}_{h}_{c}")
                           for c in range(4)] for h in range(HEADS)]
                for h in range(HEADS):
                    for c in range(4):
                        nc.vector.reciprocal(
                            rec_hc[h][c][:],
                            pv[h][:, c * 128 + 64:c * 128 + 65],
                        )
                o_cs = [o_pool.tile([128, 128], BF16, tag="o",
                                    name=f"o_{u}_{c}") for c in range(4)]
                for h in range(HEADS):
                    for c in range(4):
                        nc.vector.tensor_scalar_mul(
                            o_cs[c][:, h * 64:(h + 1) * 64],
                            pv[h][:, c * 128:c * 128 + 64],
                            rec_hc[h][c][:],
                        )
                ot = ot_pool.tile([E2, LQ], BF16, tag="ot", name=f"ot_{u}")
                ot_tiles[u] = ot
                for c in range(4):
                    tr = misc_ps.tile([128, 128], BF16, tag="tr",
                                      name=f"otr_{u}_{c}")
                    nc.tensor.transpose(tr[:], o_cs[c][:], ident[:])
                    nc.vector.tensor_copy(ot[:, c * 128:(c + 1) * 128], tr[:])

            def og(u, dt):
                """Out-projection group dt of unit u + drain to ob."""
                if dt == 0:
                    ob_tiles[u] = ob_pool.tile([128, NDT, 512], BF16,
                                               tag="ob", name=f"ob_{u}")
                op = misc_ps.tile([128, 512], F32, tag="op",
                                  name=f"og_{u}_{dt}")
                nc.tensor.matmul(
                    op[:],
                    lhsT=wo_sb[:, dt * 128:(dt + 1) * 128],
                    rhs=ot_tiles[u][:],
                    start=True, stop=True,
                )
                nc.vector.tensor_copy(ob_tiles[u][:, dt, :], op[:])

            def out_dma(u):
                b, lq = units[u]
                t0 = b * L + lq * LQ
                nc.sync.dma_start(out=out_t.ap()[:, :, t0:t0 + LQ],
                                  in_=ob_tiles[u][:])

            # ---- fill schedule: thunks run between sc quads ----
            def Fv(b, tt):
                return lambda: proj_tt("v", b, tt)

            def Fqk(n, b, tt):
                return lambda: proj_tt(n, b, tt)

            def Ftr(b, g):
                return lambda: vtr(b, g)

            def Fpv(u, g, half):
                return lambda: pv_wave(u, g, half)

            def Fta(u):
                return lambda: tail_a(u)

            def Fog(u, dt):
                return lambda: og(u, dt)

            def Fdma(u):
                return lambda: out_dma(u)

            F = {(u, g): [] for u in range(8) for g in range(4)}
            F[0, 1] = [Fv(0, 0), Ftr(0, 0)]
            F[0, 2] = [Fpv(0, 0, 0), Fv(0, 1), Fpv(0, 0, 1), Ftr(0, 1)]
            F[0, 3] = [Fqk("q", 0, 1), Fpv(0, 1, 0), Fv(0, 2),
                       Fpv(0, 1, 1), Ftr(0, 2)]
            F[1, 0] = [Fpv(0, 2, 0), Fv(0, 3), Fpv(0, 2, 1), Ftr(0, 3),
                       Fpv(0, 3, 0), Fpv(0, 3, 1), Fta(0)]
            F[1, 1] = [Fpv(1, 0, 0), Fog(0, 0), Fpv(1, 0, 1), Fog(0, 1)]
            F[1, 2] = [Fpv(1, 1, 0), Fog(0, 2), Fpv(1, 1, 1), Fog(0, 3),
                       Fqk("q", 0, 2)]
            F[1, 3] = [Fpv(1, 2, 0), Fog(0, 4), Fpv(1, 2, 1), Fog(0, 5),
                       Fqk("q", 0, 3)]
            F[2, 0] = [Fpv(1, 3, 0), Fog(0, 6), Fpv(1, 3, 1), Fog(0, 7),
                       Fdma(0), Fta(1)]
            F[2, 1] = [Fpv(2, 0, 0), Fog(1, 0), Fpv(2, 0, 1), Fog(1, 1)]
            F[2, 2] = [Fpv(2, 1, 0), Fog(1, 2), Fpv(2, 1, 1), Fog(1, 3),
                       Fqk("k", 1, 0)]
            F[2, 3] = [Fpv(2, 2, 0), Fog(1, 4), Fpv(2, 2, 1), Fog(1, 5),
                       Fqk("k", 1, 1)]
            F[3, 0] = [Fpv(2, 3, 0), Fog(1, 6), Fpv(2, 3, 1), Fog(1, 7),
                       Fdma(1), Fta(2)]
            F[3, 1] = [Fpv(3, 0, 0), Fog(2, 0), Fpv(3, 0, 1), Fog(2, 1),
                       Fqk("k", 1, 2)]
            F[3, 2] = [Fpv(3, 1, 0), Fog(2, 2), Fpv(3, 1, 1), Fog(2, 3),
                       Fqk("k", 1, 3), Fqk("q", 1, 0)]
            F[3, 3] = [Fpv(3, 2, 0), Fog(2, 4), Fpv(3, 2, 1), Fog(2, 5),
                       Fv(1, 0), Ftr(1, 0)]
            F[4, 0] = [Fpv(3, 3, 0), Fog(2, 6), Fpv(3, 3, 1), Fog(2, 7),
                       Fdma(2), Fta(3), Fv(1, 1), Ftr(1, 1)]
            F[4, 1] = [Fpv(4, 0, 0), Fog(3, 0), Fpv(4, 0, 1), Fog(3, 1),
                       Fv(1, 2), Ftr(1, 2)]
            F[4, 2] = [Fpv(4, 1, 0), Fog(3, 2), Fpv(4, 1, 1), Fog(3, 3),
                       Fv(1, 3), Ftr(1, 3), Fqk("q", 1, 1)]
            F[4, 3] = [Fpv(4, 2, 0), Fog(3, 4), Fpv(4, 2, 1), Fog(3, 5)]
            F[5, 0] = [Fpv(4, 3, 0), Fog(3, 6), Fpv(4, 3, 1), Fog(3, 7),
                       Fdma(3), Fta(4)]
            F[5, 1] = [Fpv(5, 0, 0), Fog(4, 0), Fpv(5, 0, 1), Fog(4, 1),
                       Fqk("q", 1, 2)]
            F[5, 2] = [Fpv(5, 1, 0), Fog(4, 2), Fpv(5, 1, 1), Fog(4, 3)]
            F[5, 3] = [Fpv(5, 2, 0), Fog(4, 4), Fpv(5, 2, 1), Fog(4, 5),
                       Fqk("q", 1, 3)]
            F[6, 0] = [Fpv(5, 3, 0), Fog(4, 6), Fpv(5, 3, 1), Fog(4, 7),
                       Fdma(4), Fta(5)]
            F[6, 1] = [Fpv(6, 0, 0), Fog(5, 0), Fpv(6, 0, 1), Fog(5, 1)]
            F[6, 2] = [Fpv(6, 1, 0), Fog(5, 2), Fpv(6, 1, 1), Fog(5, 3)]
            F[6, 3] = [Fpv(6, 2, 0), Fog(5, 4), Fpv(6, 2, 1), Fog(5, 5)]
            F[7, 0] = [Fpv(6, 3, 0), Fog(5, 6), Fpv(6, 3, 1), Fog(5, 7),
                       Fdma(5), Fta(6)]
            F[7, 1] = [Fpv(7, 0, 0), Fog(6, 0), Fpv(7, 0, 1), Fog(6, 1)]
            F[7, 2] = [Fpv(7, 1, 0), Fog(6, 2), Fpv(7, 1, 1), Fog(6, 3)]
            F[7, 3] = [Fpv(7, 2, 0), Fog(6, 4), Fpv(7, 2, 1), Fog(6, 5)]

            # ---- prologue projections ----
            for tt in range(4):
                proj_tt("k", 0, tt)
            proj_tt("q", 0, 0)

            # ---- main pipelined emission ----
            for u in range(8):
                for g in range(4):
                    sc_quad(u, g)
                    for thunk in F[(u, g)]:
                        thunk()

            # ---- epilogue ----
            pv_wave(7, 3, 0)
            og(6, 6)
            pv_wave(7, 3, 1)
            og(6, 7)
            out_dma(6)
            tail_a(7)
            for dt in range(NDT):
                og(7, dt)
            out_dma(7)

    nc.compile()
    return nc


def _get_nc():
    global _CACHED_NC
    if _CACHED_NC is None:
        _CACHED_NC = build_nc()
    return _CACHED_NC


def _prep_inputs(queries, keys, values, Wq, bq, Wk, bk, Wv, bv, Wo, bo):
    bf16 = ml_dtypes.bfloat16
    x_t = {}
    for n, arr in (("q", queries), ("k", keys), ("v", values)):
        # X^T [D, TOK] -> [128, NKT, TOK]: row p of kt-block kt is X^T row
        # kt*128+p
        full = np.asarray(arr, np.float32).reshape(TOK, D).T
        x_t[n] = np.ascontiguousarray(
            full.reshape(NKT, 128, TOK).transpose(1, 0, 2)
        ).astype(bf16)
    in_maps = []
    for c in range(NCORES):
        sl = slice(c * E2, (c + 1) * E2)
        m = {
            "xq_t": x_t["q"], "xk_t": x_t["k"], "xv_t": x_t["v"],
            "wq": _warrange(np.asarray(Wq, np.float32)[:, sl]),
            "wk": _warrange(np.asarray(Wk, np.float32)[:, sl]),
            "wv": _warrange(np.asarray(Wv, np.float32)[:, sl]),
            "bq": np.ascontiguousarray(np.asarray(bq, np.float32)[sl].reshape(E2, 1)),
            "bk": np.ascontiguousarray(np.asarray(bk, np.float32)[sl].reshape(E2, 1)),
            "bv": np.ascontiguousarray(np.asarray(bv, np.float32)[sl].reshape(E2, 1)),
            "wo": np.ascontiguousarray(np.asarray(Wo, np.float32)[sl, :]).astype(bf16),
            "ident_in": _IDENT,
        }
        in_maps.append(m)
    return in_maps


def _postprocess(results, bo):
    acc = np.zeros((128, NDT, TOK), np.float64)
    for r in results:
        acc += r["out_t"].astype(np.float64)  # bf16 partials, summed in fp64
    # [128 p, 8 dt, TOK] -> [D, TOK] with d = dt*128 + p
    full = acc.transpose(1, 0, 2).reshape(D, TOK)
    out = full.T.astype(np.float32) + np.asarray(bo, np.float32)[None, :]
    return out.reshape(B, L, D)


def run(trace=False, **inputs):
    nc = _get_nc()
    in_maps = _prep_inputs(**inputs)
    res = run_bass_kernel_spmd(nc, in_maps, core_ids=list(range(NCORES)),
                               trace=trace)
    out = _postprocess(res.results, inputs["bo"])
    return out, res


def kernel(**inputs):
    out, _ = run(trace=False, **inputs)
    return out


# revision 12
# speedup vs baseline: 1.2253x; 1.0872x over previous
"""Multi-head attention (B=2, L=S=2048, D=1024, H=16, E=64) on 8 TRN2 cores.

Sharding: tensor-parallel over heads. Core c owns heads 2c, 2c+1, i.e. the
128-wide slice [c*128:(c+1)*128] of the QKV projection outputs and the
matching row-slice of Wo. Each core reads the full (host-pre-transposed)
queries/keys/values, computes its two heads' attention, and writes a full
[1024, 4096] bf16 partial of the output projection; the host sums the 8
partials, transposes back and adds bo.

v2: software-pipelined emission built around the scalar-engine exp being
the pacing engine (~133us of exp vs ~140us of matmul):
  X^T  host-supplied as [128, 8 kt, 4096 tok] bf16; DMA'd per 512-tok chunk.
  QT/KT = (XW)^T        [128 e', 512]  f32r SBUF (4 tiles per batch each)
  V'_h = [V_h | 1]      [128 s, 4*65]  bf16 (PE-transposed VT)
  scores^T (per s-tile) [128 s, 2*512 (h,l)] f32 PSUM
  P^T = exp(s/8)        [128 s, 2*512] bf16 SBUF (ScalarE)
  PV flipped: lhsT=P^T chunk [128 s, 128 l], rhs=V'_h [128 s, 65]
      -> pv[h] [128 l, 4x(65 pad to 128)] PSUM accumulated over 16 s-tiles;
      col 64 of each chunk is the softmax denominator (ones column of V').
      Free-dim is 65 instead of 512, halving PE time for PV.
  normalize: per-partition reciprocal (DVE) + tensor_scalar mult (Pool)
      -> O [128 l, 128 e'] bf16; PE-transpose -> OT [128 e', 512] bf16.
  out-proj: Wo_c.T @ OT -> [128 d, 512] PSUM -> Pool copy into ob
      [128, 8, 512] bf16 -> one batched 3D DMA per unit (DVE queue).
DMA issue is kept off the scalar queue entirely so exp dispatch never
stalls behind descriptor generation. Units are emitted as 4 quads of
(8 score matmuls + 4 exps) each, with fill work (PV waves, projections,
V-transposes, previous unit's tail) interleaved between quads in an order
matched to DMA arrival times.
"""

import numpy as np
import ml_dtypes

import concourse.bass as bass
import concourse.bacc as bacc
import concourse.mybir as mybir
from concourse.tile import TileContext
from concourse.bass_utils import run_bass_kernel_spmd

BF16 = mybir.dt.bfloat16
F32 = mybir.dt.float32
F32R = mybir.dt.float32r

B, L, D = 2, 2048, 1024
TOK = B * L              # 4096
H, E = 16, 64
NCORES = 8
E2 = 128                 # projection output dims per core (2 heads)
NKT = D // 128           # 8 k-tiles of the contraction
LQ = 512                 # l-quarter: query-token tile inside attention
NLQ = L // LQ            # 4 per batch
NST = L // 128           # 16 s-tiles (key tokens) per batch
HEADS = 2                # heads per core
NDT = D // 128           # 8 output-row tiles

_CACHED_NC = None
_IDENT = np.eye(128, dtype=ml_dtypes.bfloat16)


def _warrange(w):
    # [D, E2] -> [128, NKT*E2]: row p holds [w[kt*128+p, :] for kt]
    return np.ascontiguousarray(
        w.reshape(NKT, 128, E2).transpose(1, 0, 2).reshape(128, NKT * E2)
    ).astype(ml_dtypes.bfloat16)


def build_nc():
    nc = bacc.Bacc("TRN2", target_bir_lowering=False)

    xt = {n: nc.declare_dram_parameter(f"x{n}_t", [128, NKT, TOK], BF16,
                                       isOutput=False)
          for n in ("q", "k", "v")}
    w = {n: nc.declare_dram_parameter(f"w{n}", [128, NKT * E2], BF16,
                                      isOutput=False)
         for n in ("q", "k", "v")}
    bias = {n: nc.declare_dram_parameter(f"b{n}", [E2, 1], F32, isOutput=False)
            for n in ("q", "k", "v")}
    wo = nc.declare_dram_parameter("wo", [E2, D], BF16, isOutput=False)
    ident_in = nc.declare_dram_parameter("ident_in", [128, 128], BF16,
                                         isOutput=False)
    out_t = nc.declare_dram_parameter("out_t", [128, NDT, TOK], BF16,
                                      isOutput=True)

    units = [(0, lq) for lq in range(NLQ)] + [(1, lq) for lq in range(NLQ)]

    with TileContext(nc) as tc:
        with (
            tc.tile_pool(name="const", bufs=1) as const,
            tc.tile_pool(name="persist", bufs=1) as persist,
            tc.tile_pool(name="xt_pool", bufs=10) as xt_pool,
            tc.tile_pool(name="pt_pool", bufs=20) as pt_pool,
            tc.tile_pool(name="o_pool", bufs=8) as o_pool,
            tc.tile_pool(name="rec_pool", bufs=16) as rec_pool,
            tc.tile_pool(name="ot_pool", bufs=2) as ot_pool,
            tc.tile_pool(name="ob_pool", bufs=2) as ob_pool,
            tc.tile_pool(name="sc_ps", bufs=2, space="PSUM") as sc_ps,
            tc.tile_pool(name="pv_ps", bufs=2, space="PSUM") as pv_ps,
            tc.tile_pool(name="misc_ps", bufs=1, space="PSUM") as misc_ps,
        ):
            # ---- constants ----
            ident = const.tile([128, 128], BF16, tag="ident")
            w_sb = {n: const.tile([128, NKT * E2], BF16, tag=f"w_{n}",
                                  name=f"w_{n}") for n in ("q", "k", "v")}
            b_sb = {}
            for n in ("q", "k", "v"):
                b_dma = const.tile([E2, 1], F32, tag=f"bdma_{n}",
                                   name=f"bdma_{n}")
                nc.gpsimd.dma_start(out=b_dma[:], in_=bias[n].ap())
                b_sb[n] = const.tile([E2, 1], F32, tag=f"b_{n}", name=f"b_{n}")
                nc.vector.tensor_copy(b_sb[n][:], b_dma[:])

            warm = const.tile([1, 2], F32, tag="warm")
            nc.vector.memset(warm[:], 0.0)
            nc.scalar.activation(warm[:], warm[:],
                                 mybir.ActivationFunctionType.Exp)
            wo_sb = const.tile([E2, D], BF16, tag="wo")

            qt_sbs = [[persist.tile([E2, LQ], F32R, tag=f"qt{b}_{t}",
                                    name=f"qt{b}_{t}") for t in range(4)]
                      for b in range(B)]
            kt_sbs = [[persist.tile([E2, LQ], F32R, tag=f"kt{b}_{t}",
                                    name=f"kt{b}_{t}") for t in range(4)]
                      for b in range(B)]
            vt_sbs = [[persist.tile([E2, 512], BF16, tag=f"vt{b}_{g}",
                                    name=f"vt{b}_{g}") for g in range(4)]
                      for b in range(B)]
            # V' per head: [128 spart, (4 stile, 65)] with col 64 == 1.0
            vp_sbs = [[[persist.tile([128, 4 * 65], BF16, tag=f"vp{b}_{h}_{g}",
                                     name=f"vp{b}_{h}_{g}") for g in range(4)]
                       for h in range(HEADS)] for b in range(B)]

            for b in range(B):
                for h in range(HEADS):
                    for g in range(4):
                        nc.vector.memset(vp_sbs[b][h][g][:], 1.0)

            nc.gpsimd.dma_start(out=wo_sb[:], in_=wo.ap())
            nc.gpsimd.dma_start(out=ident[:], in_=ident_in.ap())

            proj_out = {"q": qt_sbs, "k": kt_sbs}

            # ---- DMA issue (SP queue), in arrival-need order ----
            x_sb = {}

            def dma_w(n):
                nc.sync.dma_start(out=w_sb[n][:], in_=w[n].ap())

            def dma_x(n, b, tt):
                # two half-chunk DMAs into one tile: finer arrival granularity
                # at startup (DMA_ENGINES serializes transfers)
                t0 = b * L + tt * 512
                xtile = xt_pool.tile([128, NKT, 512], BF16, tag="xt",
                                     name=f"x_{n}{b}_{tt}")
                nc.sync.dma_start(out=xtile[:],
                                  in_=xt[n].ap()[:, :, t0:t0 + 512])
                x_sb[(n, b, tt)] = xtile

            dma_w("k")
            dma_w("q")
            dma_x("k", 0, 0)
            dma_x("q", 0, 0)
            dma_x("k", 0, 1)
            dma_w("v")
            dma_x("k", 0, 2)
            dma_x("k", 0, 3)
            dma_x("q", 0, 1)
            dma_x("v", 0, 0)
            dma_x("v", 0, 1)
            dma_x("v", 0, 2)
            dma_x("v", 0, 3)
            dma_x("q", 0, 2)
            dma_x("q", 0, 3)
            for key in (("k", 1, 0), ("k", 1, 1), ("k", 1, 2), ("k", 1, 3),
                        ("q", 1, 0), ("v", 1, 0), ("v", 1, 1), ("q", 1, 1),
                        ("v", 1, 2), ("v", 1, 3), ("q", 1, 2), ("q", 1, 3)):
                dma_x(*key)

            # ---- emission helpers ----
            def proj_tt(n, b, tt):
                """Project one 512-token chunk: 8 accumulating matmuls."""
                acc = misc_ps.tile([128, 512], F32, tag="op",
                                   name=f"proj_{n}{b}_{tt}")
                xtile = x_sb[(n, b, tt)]
                for kt in range(NKT):
                    nc.tensor.matmul(
                        acc[:],
                        lhsT=w_sb[n][:, kt * E2:(kt + 1) * E2],
                        rhs=xtile[:, kt, :],
                        start=(kt == 0),
                        stop=(kt == NKT - 1),
                    )
                if n == "v":
                    dst = vt_sbs[b][tt][:]
                else:
                    dst = proj_out[n][b][tt][:]
                nc.vector.tensor_scalar_add(dst, acc[:], b_sb[n][:])

            def vtr(b, g):
                """VT [128 e', s] -> V'_h [128 s, (stile, 65)] for group g."""
                for r in range(4):
                    tp = misc_ps.tile([128, 128], BF16, tag="tr",
                                      name=f"vtr_{b}_{g}_{r}")
                    nc.tensor.transpose(
                        tp[:], vt_sbs[b][g][:, r * 128:(r + 1) * 128],
                        ident[:],
                    )
                    for h in range(HEADS):
                        nc.vector.tensor_copy(
                            vp_sbs[b][h][g][:, r * 65:r * 65 + 64],
                            tp[:, h * 64:(h + 1) * 64],
                        )

            pv_tiles = {}   # u -> [h tiles]
            pt_tiles = {}   # (u, st) -> tile

            def sc_quad(u, g):
                """Scores + exp for s-tiles 4g..4g+3 of unit u."""
                b, lq = units[u]
                if g == 0:
                    pv_tiles[u] = [
                        pv_ps.tile([128, 512], F32, tag="pv",
                                   name=f"pv_{u}_{h}") for h in range(HEADS)]
                qt = qt_sbs[b][lq]
                kt = kt_sbs[b][g]
                for r in range(4):
                    st = g * 4 + r
                    sc = sc_ps.tile([128, 2 * LQ], F32, tag="sc",
                                    name=f"sc_{u}_{st}")
                    for h in range(HEADS):
                        nc.tensor.matmul(
                            sc[:, h * LQ:(h + 1) * LQ],
                            lhsT=kt[h * 64:(h + 1) * 64,
                                    r * 128:(r + 1) * 128],
                            rhs=qt[h * 64:(h + 1) * 64, :],
                            start=True, stop=True,
                        )
                    pt = pt_pool.tile([128, 2 * LQ], BF16, tag="pt",
                                      name=f"pt_{u}_{st}")
                    nc.scalar.activation(
                        pt[:], sc[:], mybir.ActivationFunctionType.Exp,
                        scale=0.125,
                    )
                    pt_tiles[(u, st)] = pt

            def pv_wave(u, g, half):
                """Flipped PV matmuls for s-tiles of quad g (half 0/1)."""
                b, lq = units[u]
                pv = pv_tiles[u]
                for r in (0, 1) if half == 0 else (2, 3):
                    st = g * 4 + r
                    pt = pt_tiles[(u, st)]
                    for h in range(HEADS):
                        for c in range(4):
                            # start resets the whole 2KB PSUM bank, so only
                            # the first write of each head-tile may set it.
                            nc.tensor.matmul(
                                pv[h][:, c * 128:c * 128 + 65],
                                lhsT=pt[:, h * LQ + c * 128:
                                        h * LQ + (c + 1) * 128],
                                rhs=vp_sbs[b][h][g][:, r * 65:(r + 1) * 65],
                                start=(st == 0 and c == 0),
                                stop=(st == NST - 1 and c == 3),
                                skip_group_check=True,
                            )

            ot_tiles = {}
            ob_tiles = {}

            def tail_a(u):
                """Reciprocals, normalize, O transpose for unit u."""
                pv = pv_tiles[u]
                rec_hc = [[rec_pool.tile([128, 1], F32, tag="rec",
                                         name=f"rec_{u}_{h}_{c}")
                           for c in range(4)] for h in range(HEADS)]
                for h in range(HEADS):
                    for c in range(4):
                        nc.vector.reciprocal(
                            rec_hc[h][c][:],
                            pv[h][:, c * 128 + 64:c * 128 + 65],
                        )
                o_cs = [o_pool.tile([128, 128], BF16, tag="o",
                                    name=f"o_{u}_{c}") for c in range(4)]
                for h in range(HEADS):
                    for c in range(4):
                        nc.vector.tensor_scalar_mul(
                            o_cs[c][:, h * 64:(h + 1) * 64],
                            pv[h][:, c * 128:c * 128 + 64],
                            rec_hc[h][c][:],
                        )
                ot = ot_pool.tile([E2, LQ], BF16, tag="ot", name=f"ot_{u}")
                ot_tiles[u] = ot
                for c in range(4):
                    tr = misc_ps.tile([128, 128], BF16, tag="tr",
                                      name=f"otr_{u}_{c}")
                    nc.tensor.transpose(tr[:], o_cs[c][:], ident[:])
                    nc.vector.tensor_copy(ot[:, c * 128:(c + 1) * 128], tr[:])

            def og(u, dt):
                """Out-projection group dt of unit u + drain to ob."""
                if dt == 0:
                    ob_tiles[u] = ob_pool.tile([128, NDT, 512], BF16,
                                               tag="ob", name=f"ob_{u}")
                op = misc_ps.tile([128, 512], F32, tag="op",
                                  name=f"og_{u}_{dt}")
                nc.tensor.matmul(
                    op[:],
                    lhsT=wo_sb[:, dt * 128:(dt + 1) * 128],
                    rhs=ot_tiles[u][:],
                    start=True, stop=True,
                )
                nc.vector.tensor_copy(ob_tiles[u][:, dt, :], op[:])

            def out_dma(u):
                b, lq = units[u]
                t0 = b * L + lq * LQ
                nc.sync.dma_start(out=out_t.ap()[:, :, t0:t0 + LQ],
                                  in_=ob_tiles[u][:])

            # ---- fill schedule: thunks run between sc quads ----
            def Fv(b, tt):
                return lambda: proj_tt("v", b, tt)

            def Fqk(n, b, tt):
                return lambda: proj_tt(n, b, tt)

            def Ftr(b, g):
                return lambda: vtr(b, g)

            def Fpv(u, g, half):
                return lambda: pv_wave(u, g, half)

            def Fta(u):
                return lambda: tail_a(u)

            def Fog(u, dt):
                return lambda: og(u, dt)

            def Fdma(u):
                return lambda: out_dma(u)

            F = {(u, g): [] for u in range(8) for g in range(4)}
            # unit 0/1: k-projections and V pipeline paced by DMA arrivals
            F[0, 0] = [Fqk("k", 0, 1)]
            F[0, 1] = [Fqk("k", 0, 2), Fqk("k", 0, 3)]
            F[0, 2] = [Fqk("q", 0, 1)]
            F[0, 3] = [Fv(0, 0), Ftr(0, 0)]
            F[1, 0] = [Fpv(0, 0, 0), Fv(0, 1), Fpv(0, 0, 1), Ftr(0, 1)]
            F[1, 1] = [Fpv(0, 1, 0), Fv(0, 2), Fpv(0, 1, 1), Ftr(0, 2)]
            F[1, 2] = [Fpv(0, 2, 0), Fv(0, 3), Fpv(0, 2, 1), Ftr(0, 3)]
            F[1, 3] = [Fpv(0, 3, 0), Fpv(0, 3, 1), Fta(0), Fqk("q", 0, 2)]
            F[2, 0] = [Fpv(1, 0, 0), Fog(0, 0), Fpv(1, 0, 1), Fog(0, 1)]
            F[2, 1] = [Fpv(1, 1, 0), Fog(0, 2), Fpv(1, 1, 1), Fog(0, 3),
                       Fqk("q", 0, 3)]
            F[2, 2] = [Fpv(1, 2, 0), Fog(0, 4), Fpv(1, 2, 1), Fog(0, 5)]
            F[2, 3] = [Fpv(1, 3, 0), Fog(0, 6), Fpv(1, 3, 1), Fog(0, 7),
                       Fdma(0), Fta(1)]
            F[3, 0] = [Fpv(2, 0, 0), Fog(1, 0), Fpv(2, 0, 1), Fog(1, 1),
                       Fqk("k", 1, 0)]
            F[3, 1] = [Fpv(2, 1, 0), Fog(1, 2), Fpv(2, 1, 1), Fog(1, 3),
                       Fqk("k", 1, 1)]
            F[3, 2] = [Fpv(2, 2, 0), Fog(1, 4), Fpv(2, 2, 1), Fog(1, 5),
                       Fqk("k", 1, 2)]
            F[3, 3] = [Fpv(2, 3, 0), Fog(1, 6), Fpv(2, 3, 1), Fog(1, 7),
                       Fdma(1), Fta(2), Fqk("k", 1, 3)]
            F[4, 0] = [Fpv(3, 0, 0), Fog(2, 0), Fpv(3, 0, 1), Fog(2, 1),
                       Fqk("q", 1, 0)]
            F[4, 1] = [Fpv(3, 1, 0), Fog(2, 2), Fpv(3, 1, 1), Fog(2, 3),
                       Fv(1, 0), Ftr(1, 0)]
            F[4, 2] = [Fpv(3, 2, 0), Fog(2, 4), Fpv(3, 2, 1), Fog(2, 5),
                       Fv(1, 1), Ftr(1, 1)]
            F[4, 3] = [Fpv(3, 3, 0), Fog(2, 6), Fpv(3, 3, 1), Fog(2, 7),
                       Fdma(2), Fta(3), Fv(1, 2), Ftr(1, 2)]
            F[5, 0] = [Fpv(4, 0, 0), Fog(3, 0), Fpv(4, 0, 1), Fog(3, 1),
                       Fv(1, 3), Ftr(1, 3)]
            F[5, 1] = [Fpv(4, 1, 0), Fog(3, 2), Fpv(4, 1, 1), Fog(3, 3),
                       Fqk("q", 1, 1)]
            F[5, 2] = [Fpv(4, 2, 0), Fog(3, 4), Fpv(4, 2, 1), Fog(3, 5)]
            F[5, 3] = [Fpv(4, 3, 0), Fog(3, 6), Fpv(4, 3, 1), Fog(3, 7),
                       Fdma(3), Fta(4)]
            F[6, 0] = [Fpv(5, 0, 0), Fog(4, 0), Fpv(5, 0, 1), Fog(4, 1),
                       Fqk("q", 1, 2)]
            F[6, 1] = [Fpv(5, 1, 0), Fog(4, 2), Fpv(5, 1, 1), Fog(4, 3)]
            F[6, 2] = [Fpv(5, 2, 0), Fog(4, 4), Fpv(5, 2, 1), Fog(4, 5),
                       Fqk("q", 1, 3)]
            F[6, 3] = [Fpv(5, 3, 0), Fog(4, 6), Fpv(5, 3, 1), Fog(4, 7),
                       Fdma(4), Fta(5)]
            F[7, 0] = [Fpv(6, 0, 0), Fog(5, 0), Fpv(6, 0, 1), Fog(5, 1)]
            F[7, 1] = [Fpv(6, 1, 0), Fog(5, 2), Fpv(6, 1, 1), Fog(5, 3)]
            F[7, 2] = [Fpv(6, 2, 0), Fog(5, 4), Fpv(6, 2, 1), Fog(5, 5)]
            F[7, 3] = [Fpv(6, 3, 0), Fog(5, 6), Fpv(6, 3, 1), Fog(5, 7),
                       Fdma(5), Fta(6)]

            # ---- prologue projections ----
            proj_tt("k", 0, 0)
            proj_tt("q", 0, 0)

            # ---- main pipelined emission ----
            for u in range(8):
                for g in range(4):
                    sc_quad(u, g)
                    for thunk in F[(u, g)]:
                        thunk()

            # ---- epilogue ----
            pv_wave(7, 0, 0)
            og(6, 0)
            pv_wave(7, 0, 1)
            og(6, 1)
            pv_wave(7, 1, 0)
            og(6, 2)
            pv_wave(7, 1, 1)
            og(6, 3)
            pv_wave(7, 2, 0)
            og(6, 4)
            pv_wave(7, 2, 1)
            og(6, 5)
            pv_wave(7, 3, 0)
            og(6, 6)
            pv_wave(7, 3, 1)
            og(6, 7)
            out_dma(6)
            tail_a(7)
            for dt in range(NDT):
                og(7, dt)
            out_dma(7)

    nc.compile()
    return nc


def _get_nc():
    global _CACHED_NC
    if _CACHED_NC is None:
        _CACHED_NC = build_nc()
    return _CACHED_NC


def _prep_inputs(queries, keys, values, Wq, bq, Wk, bk, Wv, bv, Wo, bo):
    bf16 = ml_dtypes.bfloat16
    x_t = {}
    for n, arr in (("q", queries), ("k", keys), ("v", values)):
        # X^T [D, TOK] -> [128, NKT, TOK]: row p of kt-block kt is X^T row
        # kt*128+p
        full = np.asarray(arr, np.float32).reshape(TOK, D).T
        x_t[n] = np.ascontiguousarray(
            full.reshape(NKT, 128, TOK).transpose(1, 0, 2)
        ).astype(bf16)
    in_maps = []
    for c in range(NCORES):
        sl = slice(c * E2, (c + 1) * E2)
        m = {
            "xq_t": x_t["q"], "xk_t": x_t["k"], "xv_t": x_t["v"],
            "wq": _warrange(np.asarray(Wq, np.float32)[:, sl]),
            "wk": _warrange(np.asarray(Wk, np.float32)[:, sl]),
            "wv": _warrange(np.asarray(Wv, np.float32)[:, sl]),
            "bq": np.ascontiguousarray(np.asarray(bq, np.float32)[sl].reshape(E2, 1)),
            "bk": np.ascontiguousarray(np.asarray(bk, np.float32)[sl].reshape(E2, 1)),
            "bv": np.ascontiguousarray(np.asarray(bv, np.float32)[sl].reshape(E2, 1)),
            "wo": np.ascontiguousarray(np.asarray(Wo, np.float32)[sl, :]).astype(bf16),
            "ident_in": _IDENT,
        }
        in_maps.append(m)
    return in_maps


def _postprocess(results, bo):
    acc = np.zeros((128, NDT, TOK), np.float64)
    for r in results:
        acc += r["out_t"].astype(np.float64)  # bf16 partials, summed in fp64
    # [128 p, 8 dt, TOK] -> [D, TOK] with d = dt*128 + p
    full = acc.transpose(1, 0, 2).reshape(D, TOK)
    out = full.T.astype(np.float32) + np.asarray(bo, np.float32)[None, :]
    return out.reshape(B, L, D)


def run(trace=False, **inputs):
    nc = _get_nc()
    in_maps = _prep_inputs(**inputs)
    res = run_bass_kernel_spmd(nc, in_maps, core_ids=list(range(NCORES)),
                               trace=trace)
    out = _postprocess(res.results, inputs["bo"])
    return out, res


def kernel(**inputs):
    out, _ = run(trace=False, **inputs)
    return out
